# revision 1
# baseline (speedup 1.0000x reference)
"""ChebConv (K=4) GNN layer on 8 Trainium2 NeuronCores.

Strategy (dst-sharded graph parallel):
  - Nodes are partitioned into 8 contiguous shards of 12500; core c owns all
    edges whose dst lies in its shard.
  - Each Chebyshev step s needs U = L_hat @ T_{s-1}:
      * the full T_{s-1} (node-major [N,128] f32) lives in every core's DRAM
        (x is replicated for step 1; later steps via AllGather),
      * per-core edges are grouped by (dst-group of 512, src-window of 25000)
        and gathered row-wise with dma_gather (int16 indices are
        window-relative; <=1024 indices per call, round-robin over the 4
        SWDGE queues),
      * the segment-sum over dst runs on the TensorEngine: for each 128-edge
        slice a weighted one-hot S_w[e, d] = w_e * [dstloc_e==d] (d in a
        128-wide window at a static 64-aligned base) is generated on the
        VectorEngine with two broadcast-AP tensor_tensor ops covering a whole
        gather run, and matmul(lhsT=G_slice, rhs=S_w) accumulates U^T
        (feature-major) in PSUM.
  - T'_s = 2 U - T'_{s-2} (feature-major), transposed on the TensorEngine to
    node-major for the AllGather / next gather source.
  - Output: out += c_s ⊙ (T_s @ W_s) accumulated per group; T_s @ W_s is
    computed feature-major as a W_s^T matmul then transposed; the c_s scale
    is a per-partition scalar in node-major layout.  Bias and the 8-shard
    concat happen on the host.

SPMD: one program runs on all 8 cores; all shapes/counts are static maxima
over the cores, with dummy edges (idx=0, w=0, dstloc=-1) as padding.
"""

import sys
import types

if "/opt/trn_rl_repo" not in sys.path:
    sys.path.insert(0, "/opt/trn_rl_repo")

import numpy as np


def _install_ntff_hook():
    """The image's antenv lacks axon_hooks; recreate it so trace=True works."""
    if "antenv.axon_hooks" in sys.modules:
        return
    try:
        import antenv
    except ImportError:
        return
    mod = types.ModuleType("antenv.axon_hooks")
    state = {"hook": None}
    mod.set_axon_ntff_profile_hook = lambda h: state.__setitem__("hook", h)
    mod.get_axon_ntff_profile_hook = lambda: state["hook"]
    sys.modules["antenv.axon_hooks"] = mod
    antenv.axon_hooks = mod
    try:
        from trn_agent_boot.trn_boot import _ntff_profile_via_ctypes

        mod.set_axon_ntff_profile_hook(
            _ntff_profile_via_ctypes("/opt/axon/libaxon_pjrt.so")
        )
    except Exception:
        pass


F = 128
GROUP = 512   # dst nodes per PSUM accumulation group (one f32 bank)
SUBWIN = 64   # dst sub-window granularity for static matmul bases
SW = 128      # uniform S_w width (psum slice width per matmul unit)
GCHUNK = 1024  # max indices per dma_gather call (Q7 ucode limit)
NQ = 4        # SWDGE queues


class Plan:
    __slots__ = (
        "cores", "n", "nshard", "k", "nwin", "srcwin", "ngroups", "gwidths",
        "ntiles", "runs", "total_units", "idx_cols",
        "idx", "wcol", "dstl", "xt", "call", "x_full", "weight",
    )


def _pack(x, filter_coeff, weight, edge_w, src, dst, n, cores, k, nwin):
    """Bucket/sort edges per core; build static structure + padded arrays."""
    p = Plan()
    p.cores, p.n, p.k, p.nwin = cores, n, k, nwin
    nshard = n // cores
    assert n % cores == 0
    p.nshard = nshard
    p.srcwin = (n + nwin - 1) // nwin
    assert p.srcwin <= 32768
    ngroups = (nshard + GROUP - 1) // GROUP
    p.ngroups = ngroups
    p.gwidths = [min(GROUP, nshard - g * GROUP) for g in range(ngroups)]
    p.ntiles = (nshard + 127) // 128

    src = np.asarray(src)
    dst = np.asarray(dst)
    edge_w = np.asarray(edge_w, dtype=np.float32)

    owner = dst // nshard
    dloc = dst - owner * nshard
    g_of = dloc // GROUP
    v_of = src // p.srcwin
    j_of = (dloc % GROUP) // SUBWIN
    nsub = (GROUP + SUBWIN - 1) // SUBWIN

    key = ((g_of * nwin + v_of) * nsub + j_of).astype(np.int64)
    counts = np.zeros((cores, ngroups, nwin, nsub), dtype=np.int64)
    percore = []
    for c in range(cores):
        m = owner == c
        kc = key[m]
        order = np.argsort(kc, kind="stable")
        percore.append((src[m][order], dloc[m][order], edge_w[m][order]))
        cnt = np.bincount(kc, minlength=ngroups * nwin * nsub)
        counts[c] = cnt.reshape(ngroups, nwin, nsub)

    caps = counts.max(axis=0)  # [ngroups, nwin, nsub]

    # static run/unit structure
    runs = []
    total_units = 0
    idx_cols = 0
    for g in range(ngroups):
        for v in range(nwin):
            cj = caps[g, v]
            tot = int(cj.sum())
            if tot == 0:
                continue
            c128 = (tot + 127) // 128 * 128
            pref = np.concatenate([[0], np.cumsum(cj)])
            units = []  # (s_local, base, unit_col)
            for s in range(c128 // 128):
                lo, hi = 128 * s, min(128 * s + 127, tot - 1)
                j0 = int(np.searchsorted(pref, lo, side="right") - 1)
                j1 = int(np.searchsorted(pref, hi, side="right") - 1)
                j0 = min(max(j0, 0), nsub - 1)
                j1 = min(max(j1, j0), nsub - 1)
                jb = j0
                while jb <= j1:
                    base = min(SUBWIN * jb, GROUP - SW)
                    units.append((s, base, total_units + len(units)))
                    # this unit covers windows up to base+SW
                    jcov = (base + SW) // SUBWIN - 1
                    jb = max(jcov, jb) + 1
            runs.append(
                dict(g=g, v=v, caps=cj.copy(), C=c128, idx_off=idx_cols,
                     units=units, u0=total_units)
            )
            total_units += len(units)
            idx_cols += c128 // 16
    p.runs = runs
    p.total_units = total_units
    p.idx_cols = idx_cols

    idx_all = np.zeros((cores, 128, max(idx_cols, 16)), dtype=np.int16)
    wcol = np.zeros((cores, 128, total_units), dtype=np.float32)
    dstl = np.full((cores, 128, total_units), -1.0, dtype=np.float32)

    for c in range(cores):
        sc, dc, wc = percore[c]
        cstart = np.concatenate([[0], np.cumsum(counts[c].reshape(-1))])
        for r in runs:
            g, v = r["g"], r["v"]
            C = r["C"]
            buf_src = np.zeros(C, dtype=np.int64)
            buf_dl = np.full(C, -1.0, dtype=np.float32)
            buf_w = np.zeros(C, dtype=np.float32)
            pos = 0
            for j in range(nsub):
                cap_j = int(r["caps"][j])
                if cap_j == 0:
                    continue
                bidx = (g * nwin + v) * nsub + j
                cnt_j = int(counts[c, g, v, j])
                s0 = int(cstart[bidx])
                buf_src[pos : pos + cnt_j] = sc[s0 : s0 + cnt_j]
                buf_dl[pos : pos + cnt_j] = dc[s0 : s0 + cnt_j] - g * GROUP
                buf_w[pos : pos + cnt_j] = wc[s0 : s0 + cnt_j]
                pos += cap_j
            gi = buf_src - v * p.srcwin
            gi[buf_dl < 0] = 0
            blk = gi.reshape(C // 16, 16).T.astype(np.int16)
            idx_all[c, :, r["idx_off"] : r["idx_off"] + C // 16] = np.tile(
                blk, (8, 1)
            )
            covered = np.zeros(C, dtype=bool)
            for (s, base, ucol) in r["units"]:
                seg_dl = buf_dl[128 * s : 128 * s + 128]
                seg_w = buf_w[128 * s : 128 * s + 128]
                rel = seg_dl - base
                inw = (seg_dl >= 0) & (rel >= 0) & (rel < SW)
                relx = np.where(inw, rel, -1.0).astype(np.float32)
                dstl[c, :, ucol] = relx
                wcol[c, :, ucol] = np.where(inw, seg_w, 0.0)
                covered[128 * s : 128 * s + 128] |= inw
            miss = (buf_dl >= 0) & ~covered
            assert not miss.any(), (
                f"uncovered edges in run g={g} v={v}: {miss.sum()}"
            )

    p.idx = idx_all
    p.wcol = wcol
    p.dstl = dstl

    x = np.asarray(x, dtype=np.float32)
    fc = np.asarray(filter_coeff, dtype=np.float32)
    p.x_full = np.ascontiguousarray(x)
    p.xt = np.stack(
        [
            np.ascontiguousarray(x[c * nshard : (c + 1) * nshard].T)
            for c in range(cores)
        ]
    )
    npad = p.ntiles * 128
    call = np.zeros((cores, 128, k * p.ntiles), dtype=np.float32)
    for c in range(cores):
        cc = np.zeros((k, npad), dtype=np.float32)
        cc[:, :nshard] = fc[:, c * nshard : (c + 1) * nshard]
        call[c] = (
            cc.reshape(k, p.ntiles, 128).transpose(2, 0, 1).reshape(128, -1)
        )
    p.call = call
    p.weight = np.ascontiguousarray(np.asarray(weight, dtype=np.float32))
    return p


def _build(p):
    import concourse.bacc as bacc
    import concourse.mybir as mybir
    import concourse.tile as tile

    dt = mybir.dt
    n, nshard, k = p.n, p.nshard, p.k
    ntiles, ngroups = p.ntiles, p.ngroups
    npad = ntiles * 128

    nc = bacc.Bacc(None, target_bir_lowering=False, debug=False,
                   num_devices=p.cores, num_swdge_queues=NQ)

    f32 = dt.float32
    x_full = nc.dram_tensor("x_full", [n, F], f32, kind="ExternalInput")
    xt_d = nc.dram_tensor("xt", [F, nshard], f32, kind="ExternalInput")
    w_d = nc.dram_tensor("weight", [k, F, F], f32, kind="ExternalInput")
    call_d = nc.dram_tensor("call", [128, k * ntiles], f32,
                            kind="ExternalInput")
    idx_d = nc.dram_tensor("idx", [128, max(p.idx_cols, 16)], dt.int16,
                           kind="ExternalInput")
    wcol_d = nc.dram_tensor("wcol", [128, p.total_units], f32,
                            kind="ExternalInput")
    dstl_d = nc.dram_tensor("dstl", [128, p.total_units], f32,
                            kind="ExternalInput")
    iota_d = nc.dram_tensor("iota", [128, SW], f32, kind="ExternalInput")
    ident_d = nc.dram_tensor("ident", [128, 128], f32, kind="ExternalInput")
    out_d = nc.dram_tensor("out", [npad, F], f32, kind="ExternalOutput")

    tks = [None] * k
    tkf = [None] * k
    tkp = [None] * k
    for s in range(1, k):
        if s <= k - 2:
            tks[s] = nc.dram_tensor(f"t{s}s", [nshard, F], f32)
            tkf[s] = nc.dram_tensor(f"t{s}f", [n, F], f32,
                                    addr_space="Shared")
        if s <= k - 3:
            tkp[s] = nc.dram_tensor(f"t{s}p", [F, nshard], f32)

    cmax = max((r["C"] for r in p.runs), default=128)
    umax = max((len(r["units"]) for r in p.runs), default=1)
    # S_w generation chunk (units per DVE op pair / sw tile)
    UCH = 8

    import os as _os

    max_step = int(_os.environ.get("KLIB_MAX_STEP", str(k - 1)))
    no_ag = bool(_os.environ.get("KLIB_NO_AG"))

    g_runs = {}
    for r in p.runs:
        g_runs.setdefault(r["g"], []).append(r)

    with tile.TileContext(nc) as tc:
        with (
            tc.tile_pool(name="const", bufs=1) as constp,
            tc.tile_pool(name="meta", bufs=1) as metap,
            tc.tile_pool(name="stage", bufs=2) as stagep,
            tc.tile_pool(name="sgen", bufs=4) as sgenp,
            tc.tile_pool(name="work", bufs=2) as workp,
            tc.tile_pool(name="acc", bufs=1) as accp,
            tc.tile_pool(name="psU", bufs=2, space="PSUM") as psup,
            tc.tile_pool(name="psY", bufs=2, space="PSUM") as psyp,
            tc.tile_pool(name="psT", bufs=2, space="PSUM") as pstp,
        ):
            iota_t = constp.tile([128, SW], f32)
            ident_t = constp.tile([128, 128], f32)
            wk_t = constp.tile([128, k * 128], f32)
            call_t = constp.tile([128, k * ntiles], f32)
            zeros_bf = constp.tile([128, GROUP], dt.bfloat16)
            idx_t = metap.tile([128, max(p.idx_cols, 16)], dt.int16)
            wcol_t = metap.tile([128, p.total_units], f32)
            dstl_t = metap.tile([128, p.total_units], f32)
            out_acc = accp.tile([128, npad], f32)

            nc.sync.dma_start(iota_t[:], iota_d[:])
            nc.sync.dma_start(ident_t[:], ident_d[:])
            for s in range(k):
                nc.sync.dma_start(
                    wk_t[:, s * 128 : (s + 1) * 128], w_d[s, :, :]
                )
            nc.sync.dma_start(call_t[:], call_d[:])
            nc.sync.dma_start(idx_t[:], idx_d[:])
            nc.sync.dma_start(wcol_t[:], wcol_d[:])
            nc.sync.dma_start(dstl_t[:], dstl_d[:])
            nc.gpsimd.memset(zeros_bf[:], 0.0)
            nc.vector.memset(out_acc[:], 0.0)

            def out_update(step, g, gw, src_psum):
                for i in range((gw + 127) // 128):
                    wi = min(128, gw - 128 * i)
                    t_idx = g * (GROUP // 128) + i
                    ccol = call_t[:wi, step * ntiles + t_idx
                                  : step * ntiles + t_idx + 1]
                    nc.vector.scalar_tensor_tensor(
                        out_acc[:wi, t_idx * 128 : (t_idx + 1) * 128],
                        src_psum[:wi, i * 128 : i * 128 + 128],
                        ccol,
                        out_acc[:wi, t_idx * 128 : (t_idx + 1) * 128],
                        mybir.AluOpType.mult,
                        mybir.AluOpType.add,
                    )

            # ---- step 0 ----
            for g in range(ngroups):
                gw = p.gwidths[g]
                xt_tile = workp.tile([128, GROUP], f32, tag="xt")
                nc.sync.dma_start(
                    xt_tile[:, :gw], xt_d[:, g * GROUP : g * GROUP + gw]
                )
                psY = psyp.tile([128, GROUP], f32)
                nc.tensor.matmul(
                    psY[:, :gw], wk_t[:, 0:128], xt_tile[:, :gw],
                    start=True, stop=True,
                )
                ys = workp.tile([128, GROUP], f32, tag="ys")
                nc.vector.tensor_copy(ys[:, :gw], psY[:, :gw])
                psT = pstp.tile([128, GROUP], f32)
                for i in range((gw + 127) // 128):
                    wi = min(128, gw - 128 * i)
                    nc.tensor.transpose(
                        psT[:wi, i * 128 : i * 128 + 128],
                        ys[:, i * 128 : i * 128 + wi],
                        ident_t[:],
                    )
                out_update(0, g, gw, psT)

            # ---- steps 1..k-1 ----
            n_gather = 0
            for s in range(1, min(k, max_step + 1)):
                src_full = x_full if s == 1 else tkf[s - 1]
                km2_fm = None
                if s >= 2:
                    km2_fm = xt_d if s == 2 else tkp[s - 2]

                for g in range(ngroups):
                    gw = p.gwidths[g]
                    runs_g = g_runs.get(g, [])
                    stages = []
                    for r in runs_g:
                        st = stagep.tile(
                            [128, cmax // 128, F], f32, tag=f"st{r['v']}"
                        )
                        C = r["C"]
                        for q0 in range(0, C, GCHUNK):
                            cl = min(GCHUNK, C - q0)
                            nc.gpsimd.dma_gather(
                                st[:, q0 // 128 : (q0 + cl) // 128, :],
                                src_full[r["v"] * p.srcwin
                                         : min((r["v"] + 1) * p.srcwin, n),
                                         :],
                                idx_t[:, r["idx_off"] + q0 // 16
                                      : r["idx_off"] + (q0 + cl) // 16],
                                cl, cl, F,
                                queue_num=n_gather % NQ,
                            )
                            n_gather += 1
                        stages.append(st)
                    psU = psup.tile([128, GROUP], f32)
                    n_units_g = sum(len(r["units"]) for r in runs_g)
                    nc.tensor.matmul(
                        psU[:], zeros_bf[:, :128], zeros_bf[:],
                        start=True, stop=(n_units_g == 0),
                        skip_group_check=True,
                    )
                    last_u = max(
                        (u[2] for r in runs_g for u in r["units"]),
                        default=None,
                    )
                    for r, st in zip(runs_g, stages):
                        units = r["units"]
                        for ch0 in range(0, len(units), UCH):
                            uch = units[ch0 : ch0 + UCH]
                            nu = len(uch)
                            u0 = uch[0][2]
                            sw = sgenp.tile([128, UCH, SW], f32, tag="sw")
                            iota_b = iota_t[:].rearrange(
                                "p (o w) -> p o w", o=1
                            ).broadcast_to([128, nu, SW])
                            dl_b = dstl_t[:, u0 : u0 + nu].rearrange(
                                "p (s o) -> p s o", o=1
                            ).broadcast_to([128, nu, SW])
                            w_b = wcol_t[:, u0 : u0 + nu].rearrange(
                                "p (s o) -> p s o", o=1
                            ).broadcast_to([128, nu, SW])
                            nc.vector.tensor_tensor(
                                sw[:, :nu, :], iota_b, dl_b,
                                mybir.AluOpType.is_equal,
                            )
                            nc.vector.tensor_tensor(
                                sw[:, :nu, :], sw[:, :nu, :], w_b,
                                mybir.AluOpType.mult,
                            )
                            for ju, (sl, base, ucol) in enumerate(uch):
                                nc.tensor.matmul(
                                    psU[:, base : base + SW],
                                    st[:, sl, :],
                                    sw[:, ju, :],
                                    start=False, stop=(ucol == last_u),
                                    skip_group_check=True,
                                )
                    # T' tile
                    tp = workp.tile([128, GROUP], f32, tag="tp")
                    if s == 1:
                        nc.vector.tensor_copy(tp[:, :gw], psU[:, :gw])
                    else:
                        km2 = workp.tile([128, GROUP], f32, tag="km2")
                        nc.sync.dma_start(
                            km2[:, :gw], km2_fm[:, g * GROUP : g * GROUP + gw]
                        )
                        nc.vector.scalar_tensor_tensor(
                            tp[:, :gw], psU[:, :gw], 2.0, km2[:, :gw],
                            mybir.AluOpType.mult,
                            mybir.AluOpType.subtract,
                        )
                    if tkp[s] is not None:
                        nc.sync.dma_start(
                            tkp[s][:, g * GROUP : g * GROUP + gw], tp[:, :gw]
                        )
                    # Y' = W_s^T @ T'
                    psY = psyp.tile([128, GROUP], f32)
                    nc.tensor.matmul(
                        psY[:, :gw], wk_t[:, s * 128 : s * 128 + 128],
                        tp[:, :gw], start=True, stop=True,
                    )
                    ys = workp.tile([128, GROUP], f32, tag="ys")
                    nc.vector.tensor_copy(ys[:, :gw], psY[:, :gw])
                    psT = pstp.tile([128, GROUP], f32)
                    for i in range((gw + 127) // 128):
                        wi = min(128, gw - 128 * i)
                        nc.tensor.transpose(
                            psT[:wi, i * 128 : i * 128 + 128],
                            ys[:, i * 128 : i * 128 + wi],
                            ident_t[:],
                        )
                    out_update(s, g, gw, psT)
                    # node-major T_s → shard dram for AllGather
                    if tks[s] is not None:
                        psN = pstp.tile([128, GROUP], f32, tag="psN")
                        for i in range((gw + 127) // 128):
                            wi = min(128, gw - 128 * i)
                            nc.tensor.transpose(
                                psN[:wi, i * 128 : i * 128 + 128],
                                tp[:, i * 128 : i * 128 + wi],
                                ident_t[:],
                            )
                        tn = workp.tile([128, GROUP], f32, tag="tn")
                        for i in range((gw + 127) // 128):
                            wi = min(128, gw - 128 * i)
                            nc.vector.tensor_copy(
                                tn[:wi, i * 128 : i * 128 + 128],
                                psN[:wi, i * 128 : i * 128 + 128],
                            )
                            nc.sync.dma_start(
                                tks[s][g * GROUP + i * 128
                                       : g * GROUP + i * 128 + wi, :],
                                tn[:wi, i * 128 : i * 128 + 128],
                            )
                if tks[s] is not None and not no_ag:
                    nc.gpsimd.collective_compute(
                        "AllGather",
                        mybir.AluOpType.bypass,
                        replica_groups=[list(range(p.cores))],
                        ins=[tks[s].ap().opt()],
                        outs=[tkf[s].ap().opt()],
                    )

            nc.sync.dma_start(
                out_d.ap().rearrange("(t q) f -> q t f", q=128),
                out_acc[:].rearrange("q (t f) -> q t f", f=F),
            )

    nc.compile()
    return nc


def _make_in_maps(p):
    iota = np.broadcast_to(np.arange(SW, dtype=np.float32), (128, SW)).copy()
    ident = np.eye(128, dtype=np.float32)
    maps = []
    for c in range(p.cores):
        maps.append(
            {
                "x_full": p.x_full,
                "xt": p.xt[c],
                "weight": p.weight,
                "call": p.call[c],
                "idx": p.idx[c],
                "wcol": p.wcol[c],
                "dstl": p.dstl[c],
                "iota": iota,
                "ident": ident,
            }
        )
    return maps


_LAST_EXEC_NS = None


def run(x, filter_coeff, weight, bias, edge_w, src, dst, *, cores=8, nwin=4,
        trace=False, sim=False):
    global _LAST_EXEC_NS
    n, f = np.asarray(x).shape
    assert f == F
    k = np.asarray(weight).shape[0]
    p = _pack(x, filter_coeff, weight, edge_w, src, dst, n, cores, k, nwin)
    nc = _build(p)
    in_maps = _make_in_maps(p)

    if sim:
        from concourse.bass_interp import MultiCoreSim

        msim = MultiCoreSim(nc, cores)
        for c in range(cores):
            for name, arr in in_maps[c].items():
                msim.cores[c].tensor(name)[:] = arr
        msim.simulate()
        outs = [
            np.array(msim.cores[c].mem_tensor("out")) for c in range(cores)
        ]
    else:
        _install_ntff_hook()
        from concourse import bass_utils

        res = bass_utils.run_bass_kernel_spmd(
            nc, in_maps, core_ids=list(range(cores)), trace=trace
        )
        _LAST_EXEC_NS = res.exec_time_ns
        outs = [res.results[c]["out"] for c in range(cores)]

    nshard = n // cores
    full = np.concatenate([o[:nshard] for o in outs], axis=0)
    return (full + np.asarray(bias, dtype=np.float32)[None, :]).astype(
        np.float32
    )


def kernel(x, filter_coeff, weight, bias, edge_w, src, dst):
    import os

    trace = bool(os.environ.get("KBENCH_TRACE"))
    return run(x, filter_coeff, weight, bias, edge_w, src, dst, trace=trace)


def last_exec_time_ns():
    return _LAST_EXEC_NS



# revision 2
# speedup vs baseline: 1.0593x; 1.0593x over previous
"""ChebConv (K=4) GNN layer on 8 Trainium2 NeuronCores.

Strategy (dst-sharded graph parallel), v2 — bf16 data path:
  - Nodes are partitioned into 8 contiguous shards of 12500; core c owns all
    edges whose dst lies in its shard.
  - Each Chebyshev step s needs U = L_hat @ T_{s-1}:
      * the full T_{s-1} (node-major [N,128] bf16) lives in every core's DRAM
        (x is replicated for step 1; later steps via AllGather),
      * per-core edges are grouped by (dst-group of 512, src-window) and
        gathered row-wise with dma_gather (bf16 rows of 256B; int16 indices
        are window-relative; <=1024 indices per call, round-robin over the 4
        SWDGE queues),
      * the segment-sum over dst runs on the TensorEngine: for each 128-edge
        slice a weighted one-hot S_w[e, d] = w_e * [dstloc_e==d] (d in a
        128-wide window at a static 64-aligned base) is generated on the
        VectorEngine (bf16, 2x rate) with two broadcast-AP tensor_tensor ops
        covering a whole gather run, and matmul(lhsT=G_slice, rhs=S_w) (bf16)
        accumulates U^T (feature-major) in f32 PSUM.
  - T'_s = 2 U - T'_{s-2} (feature-major, f32); T'_{s-2} comes from SBUF
    (x shard and T1 shard are SBUF-resident, no DRAM round-trip).
  - Output: out += c_s ⊙ (T_s @ W_s) accumulated per group (f32); the final
    out is dumped partition-major in ONE dma (huge contiguous packets) and
    re-laid-out on the host.  Bias and the 8-shard concat happen on the host.
  - tn (node-major T_s for the AllGather source) stores go through the
    Activation HWDGE queue; loads through the Sync queue (2 queues busy
    instead of 1).

SPMD: one program runs on all 8 cores; all shapes/counts are static maxima
over the cores, with dummy edges (idx=0, w=0, dstloc=-1) as padding.
"""

import sys
import types

if "/opt/trn_rl_repo" not in sys.path:
    sys.path.insert(0, "/opt/trn_rl_repo")

import numpy as np
import ml_dtypes

BF16 = ml_dtypes.bfloat16


def _install_ntff_hook():
    """The image's antenv lacks axon_hooks; recreate it so trace=True works."""
    if "antenv.axon_hooks" in sys.modules:
        return
    try:
        import antenv
    except ImportError:
        return
    mod = types.ModuleType("antenv.axon_hooks")
    state = {"hook": None}
    mod.set_axon_ntff_profile_hook = lambda h: state.__setitem__("hook", h)
    mod.get_axon_ntff_profile_hook = lambda: state["hook"]
    sys.modules["antenv.axon_hooks"] = mod
    antenv.axon_hooks = mod
    try:
        from trn_agent_boot.trn_boot import _ntff_profile_via_ctypes

        mod.set_axon_ntff_profile_hook(
            _ntff_profile_via_ctypes("/opt/axon/libaxon_pjrt.so")
        )
    except Exception:
        pass


F = 128
GROUP = 512   # dst nodes per PSUM accumulation group (one f32 bank)
SUBWIN = 64   # dst sub-window granularity for static matmul bases
SW = 128      # uniform S_w width (psum slice width per matmul unit)
GCHUNK = 1024  # max indices per dma_gather call (Q7 ucode limit)
NQ = 4        # SWDGE queues


class Plan:
    __slots__ = (
        "cores", "n", "nshard", "k", "nwin", "srcwin", "ngroups", "gwidths",
        "ntiles", "runs", "total_units", "idx_cols",
        "idx", "wcol", "dstl", "xt", "call", "x_full", "weight",
    )


def _pack(x, filter_coeff, weight, edge_w, src, dst, n, cores, k, nwin):
    """Bucket/sort edges per core; build static structure + padded arrays."""
    p = Plan()
    p.cores, p.n, p.k, p.nwin = cores, n, k, nwin
    nshard = n // cores
    assert n % cores == 0
    p.nshard = nshard
    p.srcwin = (n + nwin - 1) // nwin
    assert p.srcwin <= 32768
    ngroups = (nshard + GROUP - 1) // GROUP
    p.ngroups = ngroups
    p.gwidths = [min(GROUP, nshard - g * GROUP) for g in range(ngroups)]
    p.ntiles = (nshard + 127) // 128

    src = np.asarray(src)
    dst = np.asarray(dst)
    edge_w = np.asarray(edge_w, dtype=np.float32)

    owner = dst // nshard
    dloc = dst - owner * nshard
    g_of = dloc // GROUP
    v_of = src // p.srcwin
    j_of = (dloc % GROUP) // SUBWIN
    nsub = (GROUP + SUBWIN - 1) // SUBWIN

    key = ((g_of * nwin + v_of) * nsub + j_of).astype(np.int64)
    counts = np.zeros((cores, ngroups, nwin, nsub), dtype=np.int64)
    percore = []
    for c in range(cores):
        m = owner == c
        kc = key[m]
        order = np.argsort(kc, kind="stable")
        percore.append((src[m][order], dloc[m][order], edge_w[m][order]))
        cnt = np.bincount(kc, minlength=ngroups * nwin * nsub)
        counts[c] = cnt.reshape(ngroups, nwin, nsub)

    caps = counts.max(axis=0)  # [ngroups, nwin, nsub]

    # static run/unit structure
    runs = []
    total_units = 0
    idx_cols = 0
    for g in range(ngroups):
        for v in range(nwin):
            cj = caps[g, v]
            tot = int(cj.sum())
            if tot == 0:
                continue
            c128 = (tot + 127) // 128 * 128
            pref = np.concatenate([[0], np.cumsum(cj)])
            units = []  # (s_local, base, unit_col)
            for s in range(c128 // 128):
                lo, hi = 128 * s, min(128 * s + 127, tot - 1)
                j0 = int(np.searchsorted(pref, lo, side="right") - 1)
                j1 = int(np.searchsorted(pref, hi, side="right") - 1)
                j0 = min(max(j0, 0), nsub - 1)
                j1 = min(max(j1, j0), nsub - 1)
                jb = j0
                while jb <= j1:
                    base = min(SUBWIN * jb, GROUP - SW)
                    units.append((s, base, total_units + len(units)))
                    # this unit covers windows up to base+SW
                    jcov = (base + SW) // SUBWIN - 1
                    jb = max(jcov, jb) + 1
            runs.append(
                dict(g=g, v=v, caps=cj.copy(), C=c128, idx_off=idx_cols,
                     units=units, u0=total_units)
            )
            total_units += len(units)
            idx_cols += c128 // 16
    p.runs = runs
    p.total_units = total_units
    p.idx_cols = idx_cols

    idx_all = np.zeros((cores, 128, max(idx_cols, 16)), dtype=np.int16)
    wcol = np.zeros((cores, 128, total_units), dtype=np.float32)
    dstl = np.full((cores, 128, total_units), -1.0, dtype=np.float32)

    for c in range(cores):
        sc, dc, wc = percore[c]
        cstart = np.concatenate([[0], np.cumsum(counts[c].reshape(-1))])
        for r in runs:
            g, v = r["g"], r["v"]
            C = r["C"]
            buf_src = np.zeros(C, dtype=np.int64)
            buf_dl = np.full(C, -1.0, dtype=np.float32)
            buf_w = np.zeros(C, dtype=np.float32)
            pos = 0
            for j in range(nsub):
                cap_j = int(r["caps"][j])
                if cap_j == 0:
                    continue
                bidx = (g * nwin + v) * nsub + j
                cnt_j = int(counts[c, g, v, j])
                s0 = int(cstart[bidx])
                buf_src[pos : pos + cnt_j] = sc[s0 : s0 + cnt_j]
                buf_dl[pos : pos + cnt_j] = dc[s0 : s0 + cnt_j] - g * GROUP
                buf_w[pos : pos + cnt_j] = wc[s0 : s0 + cnt_j]
                pos += cap_j
            gi = buf_src - v * p.srcwin
            gi[buf_dl < 0] = 0
            blk = gi.reshape(C // 16, 16).T.astype(np.int16)
            idx_all[c, :, r["idx_off"] : r["idx_off"] + C // 16] = np.tile(
                blk, (8, 1)
            )
            covered = np.zeros(C, dtype=bool)
            for (s, base, ucol) in r["units"]:
                seg_dl = buf_dl[128 * s : 128 * s + 128]
                seg_w = buf_w[128 * s : 128 * s + 128]
                rel = seg_dl - base
                inw = (seg_dl >= 0) & (rel >= 0) & (rel < SW)
                relx = np.where(inw, rel, -1.0).astype(np.float32)
                dstl[c, :, ucol] = relx
                wcol[c, :, ucol] = np.where(inw, seg_w, 0.0)
                covered[128 * s : 128 * s + 128] |= inw
            miss = (buf_dl >= 0) & ~covered
            assert not miss.any(), (
                f"uncovered edges in run g={g} v={v}: {miss.sum()}"
            )

    p.idx = idx_all
    p.wcol = wcol.astype(BF16)
    p.dstl = dstl.astype(BF16)

    x = np.asarray(x, dtype=np.float32)
    fc = np.asarray(filter_coeff, dtype=np.float32)
    p.x_full = np.ascontiguousarray(x.astype(BF16))
    p.xt = np.stack(
        [
            np.ascontiguousarray(x[c * nshard : (c + 1) * nshard].T.astype(BF16))
            for c in range(cores)
        ]
    )
    npad = p.ntiles * 128
    call = np.zeros((cores, 128, k * p.ntiles), dtype=np.float32)
    for c in range(cores):
        cc = np.zeros((k, npad), dtype=np.float32)
        cc[:, :nshard] = fc[:, c * nshard : (c + 1) * nshard]
        call[c] = (
            cc.reshape(k, p.ntiles, 128).transpose(2, 0, 1).reshape(128, -1)
        )
    p.call = call
    p.weight = np.ascontiguousarray(np.asarray(weight, dtype=np.float32))
    return p


def _build(p):
    import concourse.bacc as bacc
    import concourse.mybir as mybir
    import concourse.tile as tile

    dt = mybir.dt
    n, nshard, k = p.n, p.nshard, p.k
    ntiles, ngroups = p.ntiles, p.ngroups
    npad = ntiles * 128

    nc = bacc.Bacc(None, target_bir_lowering=False, debug=False,
                   num_devices=p.cores, num_swdge_queues=NQ)

    f32 = dt.float32
    bf16 = dt.bfloat16
    x_full = nc.dram_tensor("x_full", [n, F], bf16, kind="ExternalInput")
    xt_d = nc.dram_tensor("xt", [F, nshard], bf16, kind="ExternalInput")
    w_d = nc.dram_tensor("weight", [k, F, F], f32, kind="ExternalInput")
    call_d = nc.dram_tensor("call", [128, k * ntiles], f32,
                            kind="ExternalInput")
    idx_d = nc.dram_tensor("idx", [128, max(p.idx_cols, 16)], dt.int16,
                           kind="ExternalInput")
    wcol_d = nc.dram_tensor("wcol", [128, p.total_units], bf16,
                            kind="ExternalInput")
    dstl_d = nc.dram_tensor("dstl", [128, p.total_units], bf16,
                            kind="ExternalInput")
    iota_d = nc.dram_tensor("iota", [128, SW], bf16, kind="ExternalInput")
    ident_d = nc.dram_tensor("ident", [128, 128], f32, kind="ExternalInput")
    out_d = nc.dram_tensor("out", [128, npad], f32, kind="ExternalOutput")

    tks = [None] * k
    tkf = [None] * k
    for s in range(1, k):
        if s <= k - 2:
            tks[s] = nc.dram_tensor(f"t{s}s", [nshard, F], bf16)
            tkf[s] = nc.dram_tensor(f"t{s}f", [n, F], bf16,
                                    addr_space="Shared")

    cmax = max((r["C"] for r in p.runs), default=128)
    # S_w generation chunk (units per DVE op pair / sw tile)
    UCH = 8

    import os as _os

    max_step = int(_os.environ.get("KLIB_MAX_STEP", str(k - 1)))
    no_ag = bool(_os.environ.get("KLIB_NO_AG"))

    g_runs = {}
    for r in p.runs:
        g_runs.setdefault(r["g"], []).append(r)

    with tile.TileContext(nc) as tc:
        with (
            tc.tile_pool(name="const", bufs=1) as constp,
            tc.tile_pool(name="meta", bufs=1) as metap,
            tc.tile_pool(name="stage", bufs=2 * p.nwin) as stagep,
            tc.tile_pool(name="sgen", bufs=4) as sgenp,
            tc.tile_pool(name="work", bufs=2) as workp,
            tc.tile_pool(name="acc", bufs=1) as accp,
            tc.tile_pool(name="psU", bufs=2, space="PSUM") as psup,
            tc.tile_pool(name="psY", bufs=2, space="PSUM") as psyp,
            tc.tile_pool(name="psT", bufs=2, space="PSUM") as pstp,
        ):
            iota_t = constp.tile([128, SW], bf16)
            ident_t = constp.tile([128, 128], f32)
            wk_t = constp.tile([128, k * 128], f32)
            wk0_bf = constp.tile([128, 128], bf16)
            call_t = constp.tile([128, k * ntiles], f32)
            zeros_bf = constp.tile([128, GROUP], dt.bfloat16)
            xslot = constp.tile([128, nshard], bf16)
            t1slot = constp.tile([128, nshard], bf16)
            idx_t = metap.tile([128, max(p.idx_cols, 16)], dt.int16)
            wcol_t = metap.tile([128, p.total_units], bf16)
            dstl_t = metap.tile([128, p.total_units], bf16)
            out_acc = accp.tile([128, npad], f32)

            nc.sync.dma_start(iota_t[:], iota_d[:])
            nc.sync.dma_start(ident_t[:], ident_d[:])
            for s in range(k):
                nc.sync.dma_start(
                    wk_t[:, s * 128 : (s + 1) * 128], w_d[s, :, :]
                )
            nc.sync.dma_start(call_t[:], call_d[:])
            nc.sync.dma_start(idx_t[:], idx_d[:])
            nc.sync.dma_start(wcol_t[:], wcol_d[:])
            nc.sync.dma_start(dstl_t[:], dstl_d[:])
            nc.scalar.dma_start(xslot[:], xt_d[:])
            nc.vector.tensor_copy(wk0_bf[:], wk_t[:, 0:128])
            nc.gpsimd.memset(zeros_bf[:], 0.0)
            nc.vector.memset(out_acc[:], 0.0)

            def out_update(step, g, gw, src_psum):
                for i in range((gw + 127) // 128):
                    wi = min(128, gw - 128 * i)
                    t_idx = g * (GROUP // 128) + i
                    ccol = call_t[:wi, step * ntiles + t_idx
                                  : step * ntiles + t_idx + 1]
                    nc.vector.scalar_tensor_tensor(
                        out_acc[:wi, t_idx * 128 : (t_idx + 1) * 128],
                        src_psum[:wi, i * 128 : i * 128 + 128],
                        ccol,
                        out_acc[:wi, t_idx * 128 : (t_idx + 1) * 128],
                        mybir.AluOpType.mult,
                        mybir.AluOpType.add,
                    )

            # ---- step 0 ----
            for g in range(ngroups):
                gw = p.gwidths[g]
                psY = psyp.tile([128, GROUP], f32)
                nc.tensor.matmul(
                    psY[:, :gw], wk0_bf[:],
                    xslot[:, g * GROUP : g * GROUP + gw],
                    start=True, stop=True,
                )
                ys = workp.tile([128, GROUP], f32, tag="ys")
                nc.vector.tensor_copy(ys[:, :gw], psY[:, :gw])
                psT = pstp.tile([128, GROUP], f32)
                for i in range((gw + 127) // 128):
                    wi = min(128, gw - 128 * i)
                    nc.tensor.transpose(
                        psT[:wi, i * 128 : i * 128 + 128],
                        ys[:, i * 128 : i * 128 + wi],
                        ident_t[:],
                    )
                out_update(0, g, gw, psT)

            # ---- steps 1..k-1 ----
            n_gather = 0
            for s in range(1, min(k, max_step + 1)):
                src_full = x_full if s == 1 else tkf[s - 1]

                for g in range(ngroups):
                    gw = p.gwidths[g]
                    runs_g = g_runs.get(g, [])
                    stages = []
                    for r in runs_g:
                        st = stagep.tile([128, cmax // 128, F], bf16)
                        C = r["C"]
                        for q0 in range(0, C, GCHUNK):
                            cl = min(GCHUNK, C - q0)
                            nc.gpsimd.dma_gather(
                                st[:, q0 // 128 : (q0 + cl) // 128, :],
                                src_full[r["v"] * p.srcwin
                                         : min((r["v"] + 1) * p.srcwin, n),
                                         :],
                                idx_t[:, r["idx_off"] + q0 // 16
                                      : r["idx_off"] + (q0 + cl) // 16],
                                cl, cl, F,
                                queue_num=n_gather % NQ,
                            )
                            n_gather += 1
                        stages.append(st)
                    psU = psup.tile([128, GROUP], f32)
                    n_units_g = sum(len(r["units"]) for r in runs_g)
                    nc.tensor.matmul(
                        psU[:], zeros_bf[:, :128], zeros_bf[:],
                        start=True, stop=(n_units_g == 0),
                        skip_group_check=True,
                    )
                    last_u = max(
                        (u[2] for r in runs_g for u in r["units"]),
                        default=None,
                    )
                    for r, st in zip(runs_g, stages):
                        units = r["units"]
                        for ch0 in range(0, len(units), UCH):
                            uch = units[ch0 : ch0 + UCH]
                            nu = len(uch)
                            u0 = uch[0][2]
                            sw = sgenp.tile([128, UCH, SW], bf16, tag="sw")
                            iota_b = iota_t[:].rearrange(
                                "p (o w) -> p o w", o=1
                            ).broadcast_to([128, nu, SW])
                            dl_b = dstl_t[:, u0 : u0 + nu].rearrange(
                                "p (s o) -> p s o", o=1
                            ).broadcast_to([128, nu, SW])
                            w_b = wcol_t[:, u0 : u0 + nu].rearrange(
                                "p (s o) -> p s o", o=1
                            ).broadcast_to([128, nu, SW])
                            nc.vector.tensor_tensor(
                                sw[:, :nu, :], iota_b, dl_b,
                                mybir.AluOpType.is_equal,
                            )
                            nc.vector.tensor_tensor(
                                sw[:, :nu, :], sw[:, :nu, :], w_b,
                                mybir.AluOpType.mult,
                            )
                            for ju, (sl, base, ucol) in enumerate(uch):
                                nc.tensor.matmul(
                                    psU[:, base : base + SW],
                                    st[:, sl, :],
                                    sw[:, ju, :],
                                    start=False, stop=(ucol == last_u),
                                    skip_group_check=True,
                                )
                    # T' tile (feature-major, f32)
                    tp = workp.tile([128, GROUP], f32, tag="tp")
                    if s == 1:
                        nc.vector.tensor_copy(tp[:, :gw], psU[:, :gw])
                        nc.vector.tensor_copy(
                            t1slot[:, g * GROUP : g * GROUP + gw],
                            tp[:, :gw],
                        )
                    else:
                        km2 = xslot if s == 2 else t1slot
                        nc.vector.scalar_tensor_tensor(
                            tp[:, :gw], psU[:, :gw], 2.0,
                            km2[:, g * GROUP : g * GROUP + gw],
                            mybir.AluOpType.mult,
                            mybir.AluOpType.subtract,
                        )
                    # Y' = W_s^T @ T'
                    psY = psyp.tile([128, GROUP], f32)
                    nc.tensor.matmul(
                        psY[:, :gw], wk_t[:, s * 128 : s * 128 + 128],
                        tp[:, :gw], start=True, stop=True,
                    )
                    ys = workp.tile([128, GROUP], f32, tag="ys")
                    nc.vector.tensor_copy(ys[:, :gw], psY[:, :gw])
                    psT = pstp.tile([128, GROUP], f32)
                    for i in range((gw + 127) // 128):
                        wi = min(128, gw - 128 * i)
                        nc.tensor.transpose(
                            psT[:wi, i * 128 : i * 128 + 128],
                            ys[:, i * 128 : i * 128 + wi],
                            ident_t[:],
                        )
                    out_update(s, g, gw, psT)
                    # node-major T_s → shard dram for AllGather
                    if tks[s] is not None:
                        psN = pstp.tile([128, GROUP], f32, tag="psN")
                        for i in range((gw + 127) // 128):
                            wi = min(128, gw - 128 * i)
                            nc.tensor.transpose(
                                psN[:wi, i * 128 : i * 128 + 128],
                                tp[:, i * 128 : i * 128 + wi],
                                ident_t[:],
                            )
                        tn = workp.tile([128, GROUP], bf16, tag="tn")
                        for i in range((gw + 127) // 128):
                            wi = min(128, gw - 128 * i)
                            nc.vector.tensor_copy(
                                tn[:wi, i * 128 : i * 128 + 128],
                                psN[:wi, i * 128 : i * 128 + 128],
                            )
                            nc.scalar.dma_start(
                                tks[s][g * GROUP + i * 128
                                       : g * GROUP + i * 128 + wi, :],
                                tn[:wi, i * 128 : i * 128 + 128],
                            )
                if tks[s] is not None and not no_ag:
                    nc.gpsimd.collective_compute(
                        "AllGather",
                        mybir.AluOpType.bypass,
                        replica_groups=[list(range(p.cores))],
                        ins=[tks[s].ap().opt()],
                        outs=[tkf[s].ap().opt()],
                    )

            nc.sync.dma_start(out_d[:], out_acc[:])

    nc.compile()
    return nc


def _make_in_maps(p):
    iota = np.broadcast_to(
        np.arange(SW, dtype=np.float32), (128, SW)
    ).astype(BF16)
    ident = np.eye(128, dtype=np.float32)
    maps = []
    for c in range(p.cores):
        maps.append(
            {
                "x_full": p.x_full,
                "xt": p.xt[c],
                "weight": p.weight,
                "call": p.call[c],
                "idx": p.idx[c],
                "wcol": p.wcol[c],
                "dstl": p.dstl[c],
                "iota": iota,
                "ident": ident,
            }
        )
    return maps


_LAST_EXEC_NS = None


def run(x, filter_coeff, weight, bias, edge_w, src, dst, *, cores=8, nwin=4,
        trace=False, sim=False):
    global _LAST_EXEC_NS
    n, f = np.asarray(x).shape
    assert f == F
    k = np.asarray(weight).shape[0]
    p = _pack(x, filter_coeff, weight, edge_w, src, dst, n, cores, k, nwin)
    nc = _build(p)
    in_maps = _make_in_maps(p)

    if sim:
        from concourse.bass_interp import MultiCoreSim

        msim = MultiCoreSim(nc, cores)
        for c in range(cores):
            for name, arr in in_maps[c].items():
                msim.cores[c].tensor(name)[:] = arr
        msim.simulate()
        outs = [
            np.array(msim.cores[c].mem_tensor("out")) for c in range(cores)
        ]
    else:
        _install_ntff_hook()
        from concourse import bass_utils

        res = bass_utils.run_bass_kernel_spmd(
            nc, in_maps, core_ids=list(range(cores)), trace=trace
        )
        _LAST_EXEC_NS = res.exec_time_ns
        outs = [res.results[c]["out"] for c in range(cores)]

    nshard = n // cores
    shards = []
    for o in outs:
        # out is [128, ntiles*128] partition-major: o[p, t*128+f] = row t*128+p
        full_pad = (
            np.asarray(o)
            .reshape(128, p.ntiles, F)
            .transpose(1, 0, 2)
            .reshape(p.ntiles * 128, F)
        )
        shards.append(full_pad[:nshard])
    full = np.concatenate(shards, axis=0)
    return (full + np.asarray(bias, dtype=np.float32)[None, :]).astype(
        np.float32
    )


def kernel(x, filter_coeff, weight, bias, edge_w, src, dst):
    import os

    trace = bool(os.environ.get("KBENCH_TRACE"))
    return run(x, filter_coeff, weight, bias, edge_w, src, dst, trace=trace)


def last_exec_time_ns():
    return _LAST_EXEC_NS


# revision 8
# speedup vs baseline: 1.0688x; 1.0090x over previous
"""ChebConv (K=4) GNN layer on 8 Trainium2 NeuronCores.

Strategy (dst-sharded graph parallel), v2 — bf16 data path:
  - Nodes are partitioned into 8 contiguous shards of 12500; core c owns all
    edges whose dst lies in its shard.
  - Each Chebyshev step s needs U = L_hat @ T_{s-1}:
      * the full T_{s-1} (node-major [N,128] bf16) lives in every core's DRAM
        (x is replicated for step 1; later steps via AllGather),
      * per-core edges are grouped by (dst-group of 512, src-window) and
        gathered row-wise with dma_gather (bf16 rows of 256B; int16 indices
        are window-relative; <=1024 indices per call, round-robin over the 4
        SWDGE queues),
      * the segment-sum over dst runs on the TensorEngine: for each 128-edge
        slice a weighted one-hot S_w[e, d] = w_e * [dstloc_e==d] (d in a
        128-wide window at a static 64-aligned base) is generated on the
        VectorEngine (bf16, 2x rate) with two broadcast-AP tensor_tensor ops
        covering a whole gather run, and matmul(lhsT=G_slice, rhs=S_w) (bf16)
        accumulates U^T (feature-major) in f32 PSUM.
  - T'_s = 2 U - T'_{s-2} (feature-major, f32); T'_{s-2} comes from SBUF
    (x shard and T1 shard are SBUF-resident, no DRAM round-trip).
  - Output: out += c_s ⊙ (T_s @ W_s) accumulated per group (f32); the final
    out is dumped partition-major in ONE dma (huge contiguous packets) and
    re-laid-out on the host.  Bias and the 8-shard concat happen on the host.
  - tn (node-major T_s for the AllGather source) stores go through the
    Activation HWDGE queue; loads through the Sync queue (2 queues busy
    instead of 1).

SPMD: one program runs on all 8 cores; all shapes/counts are static maxima
over the cores, with dummy edges (idx=0, w=0, dstloc=-1) as padding.
"""

import sys
import types

if "/opt/trn_rl_repo" not in sys.path:
    sys.path.insert(0, "/opt/trn_rl_repo")

import numpy as np
import ml_dtypes

BF16 = ml_dtypes.bfloat16


def _install_ntff_hook():
    """The image's antenv lacks axon_hooks; recreate it so trace=True works."""
    if "antenv.axon_hooks" in sys.modules:
        return
    try:
        import antenv
    except ImportError:
        return
    mod = types.ModuleType("antenv.axon_hooks")
    state = {"hook": None}
    mod.set_axon_ntff_profile_hook = lambda h: state.__setitem__("hook", h)
    mod.get_axon_ntff_profile_hook = lambda: state["hook"]
    sys.modules["antenv.axon_hooks"] = mod
    antenv.axon_hooks = mod
    try:
        from trn_agent_boot.trn_boot import _ntff_profile_via_ctypes

        mod.set_axon_ntff_profile_hook(
            _ntff_profile_via_ctypes("/opt/axon/libaxon_pjrt.so")
        )
    except Exception:
        pass


F = 128
GROUP = 512   # dst nodes per PSUM accumulation group (one f32 bank)
SUBWIN = 64   # dst sub-window granularity for static matmul bases
SW = 128      # uniform S_w width (psum slice width per matmul unit)
GCHUNK = 1024  # indices per dma_gather call
NQ = 4        # SWDGE queues
UCH = 8       # S_w generation batch (units per DVE op pair)


class Plan:
    __slots__ = (
        "cores", "n", "nshard", "k", "nwin", "srcwin", "ngroups", "gwidths",
        "ntiles", "runs", "total_units", "idx_cols",
        "idx", "wcol", "dstl", "xt", "call", "x_full", "weight",
    )


def _pack(x, filter_coeff, weight, edge_w, src, dst, n, cores, k, nwin):
    """Bucket/sort edges per core; build static structure + padded arrays."""
    p = Plan()
    p.cores, p.n, p.k, p.nwin = cores, n, k, nwin
    nshard = n // cores
    assert n % cores == 0
    p.nshard = nshard
    p.srcwin = (n + nwin - 1) // nwin
    assert p.srcwin <= 32768
    ngroups = (nshard + GROUP - 1) // GROUP
    p.ngroups = ngroups
    p.gwidths = [min(GROUP, nshard - g * GROUP) for g in range(ngroups)]
    p.ntiles = (nshard + 127) // 128

    src = np.asarray(src)
    dst = np.asarray(dst)
    edge_w = np.asarray(edge_w, dtype=np.float32)

    owner = dst // nshard
    dloc = dst - owner * nshard
    g_of = dloc // GROUP
    v_of = src // p.srcwin
    j_of = (dloc % GROUP) // SUBWIN
    nsub = (GROUP + SUBWIN - 1) // SUBWIN

    key = ((g_of * nwin + v_of) * nsub + j_of).astype(np.int64)
    counts = np.zeros((cores, ngroups, nwin, nsub), dtype=np.int64)
    percore = []
    for c in range(cores):
        m = owner == c
        kc = key[m]
        order = np.argsort(kc, kind="stable")
        percore.append((src[m][order], dloc[m][order], edge_w[m][order]))
        cnt = np.bincount(kc, minlength=ngroups * nwin * nsub)
        counts[c] = cnt.reshape(ngroups, nwin, nsub)

    caps = counts.max(axis=0)  # [ngroups, nwin, nsub]

    # static run/unit structure
    runs = []
    total_units = 0
    idx_cols = 0
    for g in range(ngroups):
        for v in range(nwin):
            cj = caps[g, v]
            tot = int(cj.sum())
            if tot == 0:
                continue
            c128 = (tot + 127) // 128 * 128
            pref = np.concatenate([[0], np.cumsum(cj)])
            units = []  # (s_local, base, unit_col)
            for s in range(c128 // 128):
                lo, hi = 128 * s, min(128 * s + 127, tot - 1)
                j0 = int(np.searchsorted(pref, lo, side="right") - 1)
                j1 = int(np.searchsorted(pref, hi, side="right") - 1)
                j0 = min(max(j0, 0), nsub - 1)
                j1 = min(max(j1, j0), nsub - 1)
                jb = j0
                while jb <= j1:
                    base = min(SUBWIN * jb, GROUP - SW)
                    units.append((s, base, total_units + len(units)))
                    # this unit covers windows up to base+SW
                    jcov = (base + SW) // SUBWIN - 1
                    jb = max(jcov, jb) + 1
            runs.append(
                dict(g=g, v=v, caps=cj.copy(), C=c128, idx_off=idx_cols,
                     units=units, u0=total_units)
            )
            total_units += len(units)
            idx_cols += c128 // 16
    p.runs = runs
    p.total_units = total_units
    p.idx_cols = idx_cols

    idx_all = np.zeros((cores, 128, max(idx_cols, 16)), dtype=np.int16)
    wcol = np.zeros((cores, 128, total_units), dtype=np.float32)
    dstl = np.full((cores, 128, total_units), -1.0, dtype=np.float32)

    for c in range(cores):
        sc, dc, wc = percore[c]
        cstart = np.concatenate([[0], np.cumsum(counts[c].reshape(-1))])
        for r in runs:
            g, v = r["g"], r["v"]
            C = r["C"]
            buf_src = np.zeros(C, dtype=np.int64)
            buf_dl = np.full(C, -1.0, dtype=np.float32)
            buf_w = np.zeros(C, dtype=np.float32)
            pos = 0
            for j in range(nsub):
                cap_j = int(r["caps"][j])
                if cap_j == 0:
                    continue
                bidx = (g * nwin + v) * nsub + j
                cnt_j = int(counts[c, g, v, j])
                s0 = int(cstart[bidx])
                buf_src[pos : pos + cnt_j] = sc[s0 : s0 + cnt_j]
                buf_dl[pos : pos + cnt_j] = dc[s0 : s0 + cnt_j] - g * GROUP
                buf_w[pos : pos + cnt_j] = wc[s0 : s0 + cnt_j]
                pos += cap_j
            gi = buf_src - v * p.srcwin
            gi[buf_dl < 0] = 0
            blk = gi.reshape(C // 16, 16).T.astype(np.int16)
            idx_all[c, :, r["idx_off"] : r["idx_off"] + C // 16] = np.tile(
                blk, (8, 1)
            )
            covered = np.zeros(C, dtype=bool)
            for (s, base, ucol) in r["units"]:
                seg_dl = buf_dl[128 * s : 128 * s + 128]
                seg_w = buf_w[128 * s : 128 * s + 128]
                rel = seg_dl - base
                inw = (seg_dl >= 0) & (rel >= 0) & (rel < SW)
                relx = np.where(inw, rel, -1.0).astype(np.float32)
                dstl[c, :, ucol] = relx
                wcol[c, :, ucol] = np.where(inw, seg_w, 0.0)
                covered[128 * s : 128 * s + 128] |= inw
            miss = (buf_dl >= 0) & ~covered
            assert not miss.any(), (
                f"uncovered edges in run g={g} v={v}: {miss.sum()}"
            )

    p.idx = idx_all
    p.wcol = wcol.astype(BF16)
    p.dstl = dstl.astype(BF16)

    x = np.asarray(x, dtype=np.float32)
    fc = np.asarray(filter_coeff, dtype=np.float32)
    p.x_full = np.ascontiguousarray(x.astype(BF16))
    p.xt = np.stack(
        [
            np.ascontiguousarray(x[c * nshard : (c + 1) * nshard].T.astype(BF16))
            for c in range(cores)
        ]
    )
    npad = p.ntiles * 128
    call = np.zeros((cores, 128, k * p.ntiles), dtype=np.float32)
    for c in range(cores):
        cc = np.zeros((k, npad), dtype=np.float32)
        cc[:, :nshard] = fc[:, c * nshard : (c + 1) * nshard]
        call[c] = (
            cc.reshape(k, p.ntiles, 128).transpose(2, 0, 1).reshape(128, -1)
        )
    p.call = call
    p.weight = np.ascontiguousarray(np.asarray(weight, dtype=np.float32))
    return p


def _build(p):
    import concourse.bacc as bacc
    import concourse.mybir as mybir
    import concourse.tile as tile

    dt = mybir.dt
    n, nshard, k = p.n, p.nshard, p.k
    ntiles, ngroups = p.ntiles, p.ngroups
    npad = ntiles * 128

    nc = bacc.Bacc(None, target_bir_lowering=False, debug=False,
                   num_devices=p.cores, num_swdge_queues=NQ)

    f32 = dt.float32
    bf16 = dt.bfloat16
    x_full = nc.dram_tensor("x_full", [n, F], bf16, kind="ExternalInput")
    xt_d = nc.dram_tensor("xt", [F, nshard], bf16, kind="ExternalInput")
    w_d = nc.dram_tensor("weight", [k, F, F], f32, kind="ExternalInput")
    call_d = nc.dram_tensor("call", [128, k * ntiles], f32,
                            kind="ExternalInput")
    idx_d = nc.dram_tensor("idx", [128, max(p.idx_cols, 16)], dt.int16,
                           kind="ExternalInput")
    wcol_d = nc.dram_tensor("wcol", [128, p.total_units], bf16,
                            kind="ExternalInput")
    dstl_d = nc.dram_tensor("dstl", [128, p.total_units], bf16,
                            kind="ExternalInput")
    iota_d = nc.dram_tensor("iota", [128, UCH * SW], bf16,
                            kind="ExternalInput")
    ident_d = nc.dram_tensor("ident", [128, 128], f32, kind="ExternalInput")
    out_d = nc.dram_tensor("out", [128, npad], f32, kind="ExternalOutput")

    tks = [None] * k
    tkf = [None] * k
    for s in range(1, k):
        if s <= k - 2:
            tks[s] = nc.dram_tensor(f"t{s}s", [nshard, F], bf16)
            tkf[s] = nc.dram_tensor(f"t{s}f", [n, F], bf16,
                                    addr_space="Shared")

    cmax = max((r["C"] for r in p.runs), default=128)

    import os as _os

    max_step = int(_os.environ.get("KLIB_MAX_STEP", str(k - 1)))
    no_ag = bool(_os.environ.get("KLIB_NO_AG"))

    g_runs = {}
    for r in p.runs:
        g_runs.setdefault(r["g"], []).append(r)

    with tile.TileContext(nc) as tc:
        with (
            tc.tile_pool(name="const", bufs=1) as constp,
            tc.tile_pool(name="meta", bufs=1) as metap,
            tc.tile_pool(name="stage", bufs=2 * p.nwin) as stagep,
            tc.tile_pool(name="sgen", bufs=4) as sgenp,
            tc.tile_pool(name="work", bufs=2) as workp,
            tc.tile_pool(name="acc", bufs=1) as accp,
            tc.tile_pool(name="psU", bufs=2, space="PSUM") as psup,
            tc.tile_pool(name="psY", bufs=2, space="PSUM") as psyp,
            tc.tile_pool(name="psT", bufs=1, space="PSUM") as pstp,
        ):
            iota_t = constp.tile([128, UCH * SW], bf16)
            ident_t = constp.tile([128, 128], f32)
            wk_t = constp.tile([128, k * 128], f32)
            wk0_bf = constp.tile([128, 128], bf16)
            call_t = constp.tile([128, k * ntiles], f32)
            zeros_bf = constp.tile([128, GROUP], dt.bfloat16)
            xslot = constp.tile([128, nshard], bf16)
            t1slot = constp.tile([128, nshard], bf16)
            idx_t = metap.tile([128, max(p.idx_cols, 16)], dt.int16)
            wcol_t = metap.tile([128, p.total_units], bf16)
            dstl_t = metap.tile([128, p.total_units], bf16)
            out_acc = accp.tile([128, npad], f32)

            nc.sync.dma_start(iota_t[:], iota_d[:])
            nc.sync.dma_start(ident_t[:], ident_d[:])
            for s in range(k):
                nc.sync.dma_start(
                    wk_t[:, s * 128 : (s + 1) * 128], w_d[s, :, :]
                )
            nc.sync.dma_start(call_t[:], call_d[:])
            nc.sync.dma_start(idx_t[:], idx_d[:])
            nc.sync.dma_start(wcol_t[:], wcol_d[:])
            nc.sync.dma_start(dstl_t[:], dstl_d[:])
            nc.scalar.dma_start(xslot[:], xt_d[:])
            nc.vector.tensor_copy(wk0_bf[:], wk_t[:, 0:128])
            nc.gpsimd.memset(zeros_bf[:], 0.0)
            nc.vector.memset(out_acc[:], 0.0)

            def out_update(step, g, gw, src_psum):
                for i in range((gw + 127) // 128):
                    wi = min(128, gw - 128 * i)
                    t_idx = g * (GROUP // 128) + i
                    ccol = call_t[:wi, step * ntiles + t_idx
                                  : step * ntiles + t_idx + 1]
                    nc.vector.scalar_tensor_tensor(
                        out_acc[:wi, t_idx * 128 : (t_idx + 1) * 128],
                        src_psum[:wi, i * 128 : i * 128 + 128],
                        ccol,
                        out_acc[:wi, t_idx * 128 : (t_idx + 1) * 128],
                        mybir.AluOpType.mult,
                        mybir.AluOpType.add,
                    )

            # ---- step 0 ----
            for g in range(ngroups):
                gw = p.gwidths[g]
                psY = psyp.tile([128, GROUP], f32)
                nc.tensor.matmul(
                    psY[:, :gw], wk0_bf[:],
                    xslot[:, g * GROUP : g * GROUP + gw],
                    start=True, stop=True,
                )
                ys = workp.tile([128, GROUP], f32, tag="ys")
                nc.vector.tensor_copy(ys[:, :gw], psY[:, :gw])
                psT = pstp.tile([128, GROUP], f32)
                for i in range((gw + 127) // 128):
                    wi = min(128, gw - 128 * i)
                    nc.tensor.transpose(
                        psT[:wi, i * 128 : i * 128 + 128],
                        ys[:, i * 128 : i * 128 + wi],
                        ident_t[:],
                    )
                out_update(0, g, gw, psT)

            # ---- steps 1..k-1 ----
            n_gather = 0
            for s in range(1, min(k, max_step + 1)):
                src_full = x_full if s == 1 else tkf[s - 1]

                for g in range(ngroups):
                    gw = p.gwidths[g]
                    runs_g = g_runs.get(g, [])
                    stages = []
                    for r in runs_g:
                        st = stagep.tile([128, cmax // 128, F], bf16)
                        C = r["C"]
                        for q0 in range(0, C, GCHUNK):
                            cl = min(GCHUNK, C - q0)
                            nc.gpsimd.dma_gather(
                                st[:, q0 // 128 : (q0 + cl) // 128, :],
                                src_full[r["v"] * p.srcwin
                                         : min((r["v"] + 1) * p.srcwin, n),
                                         :],
                                idx_t[:, r["idx_off"] + q0 // 16
                                      : r["idx_off"] + (q0 + cl) // 16],
                                cl, cl, F,
                                queue_num=n_gather % NQ,
                            )
                            n_gather += 1
                        stages.append(st)
                    psUa = psup.tile([128, GROUP], f32, tag="a")
                    psUb = psup.tile([128, GROUP], f32, tag="b")
                    units_g = [u for r in runs_g for u in r["units"]]
                    n_even = sum(1 for i in range(len(units_g)) if i % 2 == 0)
                    n_odd = len(units_g) - n_even
                    nc.tensor.matmul(
                        psUa[:], zeros_bf[:, :128], zeros_bf[:],
                        start=True, stop=(n_even == 0),
                        skip_group_check=True,
                    )
                    nc.tensor.matmul(
                        psUb[:], zeros_bf[:, :128], zeros_bf[:],
                        start=True, stop=(n_odd == 0),
                        skip_group_check=True,
                    )
                    # parity alternation avoids back-to-back PSUM
                    # accumulate hazards on the same bank
                    ug = 0
                    last_even = max(
                        (i for i in range(len(units_g)) if i % 2 == 0),
                        default=-1,
                    )
                    last_odd = max(
                        (i for i in range(len(units_g)) if i % 2 == 1),
                        default=-1,
                    )
                    for r, st in zip(runs_g, stages):
                        units = r["units"]
                        for ch0 in range(0, len(units), UCH):
                            uch = units[ch0 : ch0 + UCH]
                            nu = len(uch)
                            u0 = uch[0][2]
                            sw = sgenp.tile([128, UCH, SW], bf16, tag="sw")
                            iota_w = iota_t[:, : nu * SW].rearrange(
                                "p (s w) -> p s w", w=SW
                            )
                            dl_b = dstl_t[:, u0 : u0 + nu].rearrange(
                                "p (s o) -> p s o", o=1
                            ).broadcast_to([128, nu, SW])
                            w_b = wcol_t[:, u0 : u0 + nu].rearrange(
                                "p (s o) -> p s o", o=1
                            ).broadcast_to([128, nu, SW])
                            nc.vector.tensor_tensor(
                                sw[:, :nu, :], iota_w, dl_b,
                                mybir.AluOpType.is_equal,
                            )
                            nc.vector.tensor_tensor(
                                sw[:, :nu, :], sw[:, :nu, :], w_b,
                                mybir.AluOpType.mult,
                            )
                            for ju, (sl, base, ucol) in enumerate(uch):
                                psU = psUa if ug % 2 == 0 else psUb
                                lastu = last_even if ug % 2 == 0 else last_odd
                                nc.tensor.matmul(
                                    psU[:, base : base + SW],
                                    st[:, sl, :],
                                    sw[:, ju, :],
                                    start=False, stop=(ug == lastu),
                                    skip_group_check=True,
                                )
                                ug += 1
                    # T' tile (feature-major, f32)
                    tp = workp.tile([128, GROUP], f32, tag="tp")
                    if s == 1:
                        nc.vector.tensor_copy(tp[:, :gw], psUa[:, :gw])
                        nc.vector.tensor_tensor(
                            tp[:, :gw], psUb[:, :gw], tp[:, :gw],
                            mybir.AluOpType.add,
                        )
                        nc.vector.tensor_copy(
                            t1slot[:, g * GROUP : g * GROUP + gw],
                            tp[:, :gw],
                        )
                    else:
                        km2 = xslot if s == 2 else t1slot
                        nc.vector.scalar_tensor_tensor(
                            tp[:, :gw], psUa[:, :gw], 2.0,
                            km2[:, g * GROUP : g * GROUP + gw],
                            mybir.AluOpType.mult,
                            mybir.AluOpType.subtract,
                        )
                        nc.vector.scalar_tensor_tensor(
                            tp[:, :gw], psUb[:, :gw], 2.0,
                            tp[:, :gw],
                            mybir.AluOpType.mult,
                            mybir.AluOpType.add,
                        )
                    # Y' = W_s^T @ T'
                    psY = psyp.tile([128, GROUP], f32)
                    nc.tensor.matmul(
                        psY[:, :gw], wk_t[:, s * 128 : s * 128 + 128],
                        tp[:, :gw], start=True, stop=True,
                    )
                    ys = workp.tile([128, GROUP], f32, tag="ys")
                    nc.vector.tensor_copy(ys[:, :gw], psY[:, :gw])
                    psT = pstp.tile([128, GROUP], f32)
                    for i in range((gw + 127) // 128):
                        wi = min(128, gw - 128 * i)
                        nc.tensor.transpose(
                            psT[:wi, i * 128 : i * 128 + 128],
                            ys[:, i * 128 : i * 128 + wi],
                            ident_t[:],
                        )
                    out_update(s, g, gw, psT)
                    # node-major T_s → shard dram for AllGather
                    if tks[s] is not None:
                        psN = pstp.tile([128, GROUP], f32, tag="psN")
                        for i in range((gw + 127) // 128):
                            wi = min(128, gw - 128 * i)
                            nc.tensor.transpose(
                                psN[:wi, i * 128 : i * 128 + 128],
                                tp[:, i * 128 : i * 128 + wi],
                                ident_t[:],
                            )
                        tn = workp.tile([128, GROUP], bf16, tag="tn")
                        if gw == GROUP:
                            nc.vector.tensor_copy(tn[:], psN[:])
                        else:
                            for i in range((gw + 127) // 128):
                                wi = min(128, gw - 128 * i)
                                nc.vector.tensor_copy(
                                    tn[:wi, i * 128 : i * 128 + 128],
                                    psN[:wi, i * 128 : i * 128 + 128],
                                )
                        for i in range((gw + 127) // 128):
                            wi = min(128, gw - 128 * i)
                            nc.scalar.dma_start(
                                tks[s][g * GROUP + i * 128
                                       : g * GROUP + i * 128 + wi, :],
                                tn[:wi, i * 128 : i * 128 + 128],
                            )
                if tks[s] is not None and not no_ag:
                    nc.gpsimd.collective_compute(
                        "AllGather",
                        mybir.AluOpType.bypass,
                        replica_groups=[list(range(p.cores))],
                        ins=[tks[s].ap().opt()],
                        outs=[tkf[s].ap().opt()],
                    )

            nc.sync.dma_start(out_d[:], out_acc[:])

    nc.compile()
    return nc


def _make_in_maps(p):
    iota = np.broadcast_to(
        np.tile(np.arange(SW, dtype=np.float32), UCH), (128, UCH * SW)
    ).astype(BF16)
    ident = np.eye(128, dtype=np.float32)
    maps = []
    for c in range(p.cores):
        maps.append(
            {
                "x_full": p.x_full,
                "xt": p.xt[c],
                "weight": p.weight,
                "call": p.call[c],
                "idx": p.idx[c],
                "wcol": p.wcol[c],
                "dstl": p.dstl[c],
                "iota": iota,
                "ident": ident,
            }
        )
    return maps


_LAST_EXEC_NS = None


def run(x, filter_coeff, weight, bias, edge_w, src, dst, *, cores=8, nwin=4,
        trace=False, sim=False):
    global _LAST_EXEC_NS
    n, f = np.asarray(x).shape
    assert f == F
    k = np.asarray(weight).shape[0]
    p = _pack(x, filter_coeff, weight, edge_w, src, dst, n, cores, k, nwin)
    nc = _build(p)
    in_maps = _make_in_maps(p)

    if sim:
        from concourse.bass_interp import MultiCoreSim

        msim = MultiCoreSim(nc, cores)
        for c in range(cores):
            for name, arr in in_maps[c].items():
                msim.cores[c].tensor(name)[:] = arr
        msim.simulate()
        outs = [
            np.array(msim.cores[c].mem_tensor("out")) for c in range(cores)
        ]
    else:
        _install_ntff_hook()
        from concourse import bass_utils

        res = bass_utils.run_bass_kernel_spmd(
            nc, in_maps, core_ids=list(range(cores)), trace=trace
        )
        _LAST_EXEC_NS = res.exec_time_ns
        outs = [res.results[c]["out"] for c in range(cores)]

    nshard = n // cores
    shards = []
    for o in outs:
        # out is [128, ntiles*128] partition-major: o[p, t*128+f] = row t*128+p
        full_pad = (
            np.asarray(o)
            .reshape(128, p.ntiles, F)
            .transpose(1, 0, 2)
            .reshape(p.ntiles * 128, F)
        )
        shards.append(full_pad[:nshard])
    full = np.concatenate(shards, axis=0)
    return (full + np.asarray(bias, dtype=np.float32)[None, :]).astype(
        np.float32
    )


def kernel(x, filter_coeff, weight, bias, edge_w, src, dst):
    import os

    trace = bool(os.environ.get("KBENCH_TRACE"))
    return run(x, filter_coeff, weight, bias, edge_w, src, dst, trace=trace)


def last_exec_time_ns():
    return _LAST_EXEC_NS


# revision 10
# speedup vs baseline: 1.4564x; 1.3626x over previous
"""ChebConv (K=4) GNN layer on 8 Trainium2 NeuronCores.

Strategy (dst-sharded graph parallel), v2 — bf16 data path:
  - Nodes are partitioned into 8 contiguous shards of 12500; core c owns all
    edges whose dst lies in its shard.
  - Each Chebyshev step s needs U = L_hat @ T_{s-1}:
      * the full T_{s-1} (node-major [N,128] bf16) lives in every core's DRAM
        (x is replicated for step 1; later steps via AllGather),
      * per-core edges are grouped by (dst-group of 512, src-window) and
        gathered row-wise with dma_gather (bf16 rows of 256B; int16 indices
        are window-relative; <=1024 indices per call, round-robin over the 4
        SWDGE queues),
      * the segment-sum over dst runs on the TensorEngine: for each 128-edge
        slice a weighted one-hot S_w[e, d] = w_e * [dstloc_e==d] (d in a
        128-wide window at a static 64-aligned base) is generated on the
        VectorEngine (bf16, 2x rate) with two broadcast-AP tensor_tensor ops
        covering a whole gather run, and matmul(lhsT=G_slice, rhs=S_w) (bf16)
        accumulates U^T (feature-major) in f32 PSUM.
  - T'_s = 2 U - T'_{s-2} (feature-major, f32); T'_{s-2} comes from SBUF
    (x shard and T1 shard are SBUF-resident, no DRAM round-trip).
  - Output: out += c_s ⊙ (T_s @ W_s) accumulated per group (f32); the final
    out is dumped partition-major in ONE dma (huge contiguous packets) and
    re-laid-out on the host.  Bias and the 8-shard concat happen on the host.
  - tn (node-major T_s for the AllGather source) stores go through the
    Activation HWDGE queue; loads through the Sync queue (2 queues busy
    instead of 1).

SPMD: one program runs on all 8 cores; all shapes/counts are static maxima
over the cores, with dummy edges (idx=0, w=0, dstloc=-1) as padding.
"""

import sys
import types

if "/opt/trn_rl_repo" not in sys.path:
    sys.path.insert(0, "/opt/trn_rl_repo")

import numpy as np
import ml_dtypes

BF16 = ml_dtypes.bfloat16


def _install_ntff_hook():
    """The image's antenv lacks axon_hooks; recreate it so trace=True works."""
    if "antenv.axon_hooks" in sys.modules:
        return
    try:
        import antenv
    except ImportError:
        return
    mod = types.ModuleType("antenv.axon_hooks")
    state = {"hook": None}
    mod.set_axon_ntff_profile_hook = lambda h: state.__setitem__("hook", h)
    mod.get_axon_ntff_profile_hook = lambda: state["hook"]
    sys.modules["antenv.axon_hooks"] = mod
    antenv.axon_hooks = mod
    try:
        from trn_agent_boot.trn_boot import _ntff_profile_via_ctypes

        mod.set_axon_ntff_profile_hook(
            _ntff_profile_via_ctypes("/opt/axon/libaxon_pjrt.so")
        )
    except Exception:
        pass


F = 128
GROUP = 512   # dst nodes per PSUM accumulation group (one f32 bank)
SUBWIN = 64   # dst sub-window granularity for static matmul bases
SW = 128      # uniform S_w width (psum slice width per matmul unit)
GCHUNK = 1024  # indices per dma_gather call (2048+ fails at runtime)
NQ = 4        # SWDGE queues
UCH = 8       # S_w generation batch (units per DVE op pair)


class Plan:
    __slots__ = (
        "cores", "n", "nshard", "k", "nwin", "srcwin", "ngroups", "gwidths",
        "ntiles", "runs", "total_units", "idx_cols",
        "idx", "wcol", "dstl_hi", "dstl_lo", "xt", "call", "x_full",
        "weight",
    )


def _pack(x, filter_coeff, weight, edge_w, src, dst, n, cores, k, nwin):
    """Bucket/sort edges per core; build static structure + padded arrays."""
    p = Plan()
    p.cores, p.n, p.k, p.nwin = cores, n, k, nwin
    nshard = n // cores
    assert n % cores == 0
    p.nshard = nshard
    p.srcwin = (n + nwin - 1) // nwin
    assert p.srcwin <= 32768
    ngroups = (nshard + GROUP - 1) // GROUP
    p.ngroups = ngroups
    p.gwidths = [min(GROUP, nshard - g * GROUP) for g in range(ngroups)]
    p.ntiles = (nshard + 127) // 128

    src = np.asarray(src)
    dst = np.asarray(dst)
    edge_w = np.asarray(edge_w, dtype=np.float32)

    owner = dst // nshard
    dloc = dst - owner * nshard
    g_of = dloc // GROUP
    v_of = src // p.srcwin
    j_of = (dloc % GROUP) // SUBWIN
    nsub = (GROUP + SUBWIN - 1) // SUBWIN

    key = ((g_of * nwin + v_of) * nsub + j_of).astype(np.int64)
    counts = np.zeros((cores, ngroups, nwin, nsub), dtype=np.int64)
    percore = []
    for c in range(cores):
        m = owner == c
        kc = key[m]
        order = np.argsort(kc, kind="stable")
        percore.append((src[m][order], dloc[m][order], edge_w[m][order]))
        cnt = np.bincount(kc, minlength=ngroups * nwin * nsub)
        counts[c] = cnt.reshape(ngroups, nwin, nsub)

    caps = counts.max(axis=0)  # [ngroups, nwin, nsub]

    # static run/unit structure
    runs = []
    total_units = 0
    idx_cols = 0
    for g in range(ngroups):
        for v in range(nwin):
            cj = caps[g, v]
            tot = int(cj.sum())
            if tot == 0:
                continue
            c128 = (tot + 127) // 128 * 128
            pref = np.concatenate([[0], np.cumsum(cj)])
            units = []  # (s_local, base, unit_col)
            for s in range(c128 // 128):
                lo, hi = 128 * s, min(128 * s + 127, tot - 1)
                j0 = int(np.searchsorted(pref, lo, side="right") - 1)
                j1 = int(np.searchsorted(pref, hi, side="right") - 1)
                j0 = min(max(j0, 0), nsub - 1)
                j1 = min(max(j1, j0), nsub - 1)
                jb = j0
                while jb <= j1:
                    base = min(SUBWIN * jb, GROUP - SW)
                    units.append((s, base, total_units + len(units)))
                    # this unit covers windows up to base+SW
                    jcov = (base + SW) // SUBWIN - 1
                    jb = max(jcov, jb) + 1
            runs.append(
                dict(g=g, v=v, caps=cj.copy(), C=c128, idx_off=idx_cols,
                     units=units, u0=total_units)
            )
            total_units += len(units)
            idx_cols += c128 // 16
    p.runs = runs
    p.total_units = total_units
    p.idx_cols = idx_cols

    idx_all = np.zeros((cores, 128, max(idx_cols, 16)), dtype=np.int16)
    wcol = np.zeros((cores, 128, total_units), dtype=np.float32)
    dstl = np.full((cores, 128, total_units), -1.0, dtype=np.float32)

    for c in range(cores):
        sc, dc, wc = percore[c]
        cstart = np.concatenate([[0], np.cumsum(counts[c].reshape(-1))])
        for r in runs:
            g, v = r["g"], r["v"]
            C = r["C"]
            buf_src = np.zeros(C, dtype=np.int64)
            buf_dl = np.full(C, -1.0, dtype=np.float32)
            buf_w = np.zeros(C, dtype=np.float32)
            pos = 0
            for j in range(nsub):
                cap_j = int(r["caps"][j])
                if cap_j == 0:
                    continue
                bidx = (g * nwin + v) * nsub + j
                cnt_j = int(counts[c, g, v, j])
                s0 = int(cstart[bidx])
                buf_src[pos : pos + cnt_j] = sc[s0 : s0 + cnt_j]
                buf_dl[pos : pos + cnt_j] = dc[s0 : s0 + cnt_j] - g * GROUP
                buf_w[pos : pos + cnt_j] = wc[s0 : s0 + cnt_j]
                pos += cap_j
            gi = buf_src - v * p.srcwin
            gi[buf_dl < 0] = 0
            blk = gi.reshape(C // 16, 16).T.astype(np.int16)
            idx_all[c, :, r["idx_off"] : r["idx_off"] + C // 16] = np.tile(
                blk, (8, 1)
            )
            covered = np.zeros(C, dtype=bool)
            for (s, base, ucol) in r["units"]:
                seg_dl = buf_dl[128 * s : 128 * s + 128]
                seg_w = buf_w[128 * s : 128 * s + 128]
                rel = seg_dl - base
                inw = (seg_dl >= 0) & (rel >= 0) & (rel < SW)
                relx = np.where(inw, rel, -1.0).astype(np.float32)
                dstl[c, :, ucol] = relx
                wcol[c, :, ucol] = np.where(inw, seg_w, 0.0)
                covered[128 * s : 128 * s + 128] |= inw
            miss = (buf_dl >= 0) & ~covered
            assert not miss.any(), (
                f"uncovered edges in run g={g} v={v}: {miss.sum()}"
            )

    p.idx = idx_all
    p.wcol = wcol.astype(BF16)
    dh = np.floor(dstl / 8.0)
    p.dstl_hi = dh.astype(BF16)
    p.dstl_lo = (dstl - 8.0 * dh).astype(BF16)

    x = np.asarray(x, dtype=np.float32)
    fc = np.asarray(filter_coeff, dtype=np.float32)
    p.x_full = np.ascontiguousarray(x.astype(BF16))
    p.xt = np.stack(
        [
            np.ascontiguousarray(x[c * nshard : (c + 1) * nshard].T.astype(BF16))
            for c in range(cores)
        ]
    )
    npad = p.ntiles * 128
    call = np.zeros((cores, 128, k * p.ntiles), dtype=np.float32)
    for c in range(cores):
        cc = np.zeros((k, npad), dtype=np.float32)
        cc[:, :nshard] = fc[:, c * nshard : (c + 1) * nshard]
        call[c] = (
            cc.reshape(k, p.ntiles, 128).transpose(2, 0, 1).reshape(128, -1)
        )
    p.call = call
    p.weight = np.ascontiguousarray(np.asarray(weight, dtype=np.float32))
    return p


def _build(p):
    import concourse.bacc as bacc
    import concourse.mybir as mybir
    import concourse.tile as tile

    dt = mybir.dt
    n, nshard, k = p.n, p.nshard, p.k
    ntiles, ngroups = p.ntiles, p.ngroups
    npad = ntiles * 128

    nc = bacc.Bacc(None, target_bir_lowering=False, debug=False,
                   num_devices=p.cores, num_swdge_queues=NQ)

    f32 = dt.float32
    bf16 = dt.bfloat16
    x_full = nc.dram_tensor("x_full", [n, F], bf16, kind="ExternalInput")
    xt_d = nc.dram_tensor("xt", [F, nshard], bf16, kind="ExternalInput")
    w_d = nc.dram_tensor("weight", [k, F, F], f32, kind="ExternalInput")
    call_d = nc.dram_tensor("call", [128, k * ntiles], f32,
                            kind="ExternalInput")
    idx_d = nc.dram_tensor("idx", [128, max(p.idx_cols, 16)], dt.int16,
                           kind="ExternalInput")
    wcol_d = nc.dram_tensor("wcol", [128, p.total_units], bf16,
                            kind="ExternalInput")
    dsthi_d = nc.dram_tensor("dstl_hi", [128, p.total_units], bf16,
                             kind="ExternalInput")
    dstlo_d = nc.dram_tensor("dstl_lo", [128, p.total_units], bf16,
                             kind="ExternalInput")
    iotah_d = nc.dram_tensor("iota_hi", [128, UCH * 16], bf16,
                             kind="ExternalInput")
    iotal_d = nc.dram_tensor("iota_lo", [128, UCH * 8], bf16,
                             kind="ExternalInput")
    ident_d = nc.dram_tensor("ident", [128, 128], f32, kind="ExternalInput")
    out_d = nc.dram_tensor("out", [128, npad], f32, kind="ExternalOutput")

    tks = [None] * k
    tkf = [None] * k
    for s in range(1, k):
        if s <= k - 2:
            tks[s] = nc.dram_tensor(f"t{s}s", [nshard, F], bf16)
            tkf[s] = nc.dram_tensor(f"t{s}f", [n, F], bf16,
                                    addr_space="Shared")

    cmax = max((r["C"] for r in p.runs), default=128)

    import os as _os

    max_step = int(_os.environ.get("KLIB_MAX_STEP", str(k - 1)))
    no_ag = bool(_os.environ.get("KLIB_NO_AG"))

    g_runs = {}
    for r in p.runs:
        g_runs.setdefault(r["g"], []).append(r)

    with tile.TileContext(nc) as tc:
        with (
            tc.tile_pool(name="const", bufs=1) as constp,
            tc.tile_pool(name="meta", bufs=1) as metap,
            tc.tile_pool(name="stage", bufs=2 * p.nwin) as stagep,
            tc.tile_pool(name="sgen", bufs=4) as sgenp,
            tc.tile_pool(name="work", bufs=2) as workp,
            tc.tile_pool(name="acc", bufs=1) as accp,
            tc.tile_pool(name="psU", bufs=2, space="PSUM") as psup,
            tc.tile_pool(name="psY", bufs=2, space="PSUM") as psyp,
            tc.tile_pool(name="psT", bufs=1, space="PSUM") as pstp,
        ):
            iotah_t = constp.tile([128, UCH * 16], bf16)
            iotal_t = constp.tile([128, UCH * 8], bf16)
            ident_t = constp.tile([128, 128], f32)
            wk_t = constp.tile([128, k * 128], f32)
            wk0_bf = constp.tile([128, 128], bf16)
            call_t = constp.tile([128, k * ntiles], f32)
            zeros_bf = constp.tile([128, GROUP], dt.bfloat16)
            xslot = constp.tile([128, nshard], bf16)
            t1slot = constp.tile([128, nshard], bf16)
            idx_t = metap.tile([128, max(p.idx_cols, 16)], dt.int16)
            wcol_t = metap.tile([128, p.total_units], bf16)
            dsthi_t = metap.tile([128, p.total_units], bf16)
            dstlo_t = metap.tile([128, p.total_units], bf16)
            out_acc = accp.tile([128, npad], f32)

            nc.sync.dma_start(iotah_t[:], iotah_d[:])
            nc.sync.dma_start(iotal_t[:], iotal_d[:])
            nc.sync.dma_start(ident_t[:], ident_d[:])
            for s in range(k):
                nc.sync.dma_start(
                    wk_t[:, s * 128 : (s + 1) * 128], w_d[s, :, :]
                )
            nc.sync.dma_start(call_t[:], call_d[:])
            nc.sync.dma_start(idx_t[:], idx_d[:])
            nc.sync.dma_start(wcol_t[:], wcol_d[:])
            nc.sync.dma_start(dsthi_t[:], dsthi_d[:])
            nc.sync.dma_start(dstlo_t[:], dstlo_d[:])
            nc.scalar.dma_start(xslot[:], xt_d[:])
            nc.vector.tensor_copy(wk0_bf[:], wk_t[:, 0:128])
            nc.gpsimd.memset(zeros_bf[:], 0.0)
            nc.vector.memset(out_acc[:], 0.0)

            def out_update(step, g, gw, src_psum):
                for i in range((gw + 127) // 128):
                    wi = min(128, gw - 128 * i)
                    t_idx = g * (GROUP // 128) + i
                    ccol = call_t[:wi, step * ntiles + t_idx
                                  : step * ntiles + t_idx + 1]
                    nc.vector.scalar_tensor_tensor(
                        out_acc[:wi, t_idx * 128 : (t_idx + 1) * 128],
                        src_psum[:wi, i * 128 : i * 128 + 128],
                        ccol,
                        out_acc[:wi, t_idx * 128 : (t_idx + 1) * 128],
                        mybir.AluOpType.mult,
                        mybir.AluOpType.add,
                    )

            # ---- step 0 ----
            for g in range(ngroups):
                gw = p.gwidths[g]
                psY = psyp.tile([128, GROUP], f32)
                nc.tensor.matmul(
                    psY[:, :gw], wk0_bf[:],
                    xslot[:, g * GROUP : g * GROUP + gw],
                    start=True, stop=True,
                )
                ys = workp.tile([128, GROUP], f32, tag="ys")
                nc.scalar.activation(
                    ys[:, :gw], psY[:, :gw],
                    mybir.ActivationFunctionType.Copy,
                )
                psT = pstp.tile([128, GROUP], f32)
                for i in range((gw + 127) // 128):
                    wi = min(128, gw - 128 * i)
                    nc.tensor.transpose(
                        psT[:wi, i * 128 : i * 128 + 128],
                        ys[:, i * 128 : i * 128 + wi],
                        ident_t[:],
                    )
                out_update(0, g, gw, psT)

            # ---- steps 1..k-1 ----
            n_gather = 0
            for s in range(1, min(k, max_step + 1)):
                src_full = x_full if s == 1 else tkf[s - 1]

                for g in range(ngroups):
                    gw = p.gwidths[g]
                    runs_g = g_runs.get(g, [])
                    stages = []
                    for r in runs_g:
                        st = stagep.tile([128, cmax // 128, F], bf16)
                        C = r["C"]
                        for q0 in range(0, C, GCHUNK):
                            cl = min(GCHUNK, C - q0)
                            nc.gpsimd.dma_gather(
                                st[:, q0 // 128 : (q0 + cl) // 128, :],
                                src_full[r["v"] * p.srcwin
                                         : min((r["v"] + 1) * p.srcwin, n),
                                         :],
                                idx_t[:, r["idx_off"] + q0 // 16
                                      : r["idx_off"] + (q0 + cl) // 16],
                                cl, cl, F,
                                queue_num=n_gather % NQ,
                            )
                            n_gather += 1
                        stages.append(st)
                    psUa = psup.tile([128, GROUP], f32, tag="a")
                    psUb = psup.tile([128, GROUP], f32, tag="b")
                    units_g = [u for r in runs_g for u in r["units"]]
                    n_even = sum(1 for i in range(len(units_g)) if i % 2 == 0)
                    n_odd = len(units_g) - n_even
                    nc.tensor.matmul(
                        psUa[:], zeros_bf[:, :128], zeros_bf[:],
                        start=True, stop=(n_even == 0),
                        skip_group_check=True,
                    )
                    nc.tensor.matmul(
                        psUb[:], zeros_bf[:, :128], zeros_bf[:],
                        start=True, stop=(n_odd == 0),
                        skip_group_check=True,
                    )
                    # parity alternation avoids back-to-back PSUM
                    # accumulate hazards on the same bank
                    ug = 0
                    last_even = max(
                        (i for i in range(len(units_g)) if i % 2 == 0),
                        default=-1,
                    )
                    last_odd = max(
                        (i for i in range(len(units_g)) if i % 2 == 1),
                        default=-1,
                    )
                    for r, st in zip(runs_g, stages):
                        units = r["units"]
                        for ch0 in range(0, len(units), UCH):
                            uch = units[ch0 : ch0 + UCH]
                            nu = len(uch)
                            u0 = uch[0][2]
                            sw = sgenp.tile([128, UCH, SW], bf16, tag="sw")
                            ohh = sgenp.tile([128, UCH, 16], bf16, tag="oh")
                            ohl = sgenp.tile([128, UCH, 8], bf16, tag="ol")
                            dlh_b = dsthi_t[:, u0 : u0 + nu].rearrange(
                                "p (s o) -> p s o", o=1
                            ).broadcast_to([128, nu, 16])
                            dll_b = dstlo_t[:, u0 : u0 + nu].rearrange(
                                "p (s o) -> p s o", o=1
                            ).broadcast_to([128, nu, 8])
                            w_b = wcol_t[:, u0 : u0 + nu].rearrange(
                                "p (s o) -> p s o", o=1
                            ).broadcast_to([128, nu, 16])
                            nc.vector.tensor_tensor(
                                ohh[:, :nu, :],
                                iotah_t[:, : nu * 16].rearrange(
                                    "p (s w) -> p s w", w=16
                                ),
                                dlh_b, mybir.AluOpType.is_equal,
                            )
                            nc.vector.tensor_tensor(
                                ohh[:, :nu, :], ohh[:, :nu, :], w_b,
                                mybir.AluOpType.mult,
                            )
                            nc.vector.tensor_tensor(
                                ohl[:, :nu, :],
                                iotal_t[:, : nu * 8].rearrange(
                                    "p (s w) -> p s w", w=8
                                ),
                                dll_b, mybir.AluOpType.is_equal,
                            )
                            ohh_b = ohh[:, :nu, :].rearrange(
                                "p s (h o) -> p s h o", o=1
                            ).broadcast_to([128, nu, 16, 8])
                            ohl_b = ohl[:, :nu, :].rearrange(
                                "p s (o l) -> p s o l", o=1
                            ).broadcast_to([128, nu, 16, 8])
                            nc.vector.tensor_tensor(
                                sw[:, :nu, :].rearrange(
                                    "p s (h l) -> p s h l", l=8
                                ),
                                ohh_b, ohl_b,
                                mybir.AluOpType.mult,
                            )
                            for ju, (sl, base, ucol) in enumerate(uch):
                                psU = psUa if ug % 2 == 0 else psUb
                                lastu = last_even if ug % 2 == 0 else last_odd
                                nc.tensor.matmul(
                                    psU[:, base : base + SW],
                                    st[:, sl, :],
                                    sw[:, ju, :],
                                    start=False, stop=(ug == lastu),
                                    skip_group_check=True,
                                )
                                ug += 1
                    # T' tile (feature-major, f32)
                    tp = workp.tile([128, GROUP], f32, tag="tp")
                    if s == 1:
                        nc.vector.tensor_copy(tp[:, :gw], psUa[:, :gw])
                        nc.vector.tensor_tensor(
                            tp[:, :gw], psUb[:, :gw], tp[:, :gw],
                            mybir.AluOpType.add,
                        )
                        nc.vector.tensor_copy(
                            t1slot[:, g * GROUP : g * GROUP + gw],
                            tp[:, :gw],
                        )
                    else:
                        km2 = xslot if s == 2 else t1slot
                        nc.vector.scalar_tensor_tensor(
                            tp[:, :gw], psUa[:, :gw], 2.0,
                            km2[:, g * GROUP : g * GROUP + gw],
                            mybir.AluOpType.mult,
                            mybir.AluOpType.subtract,
                        )
                        nc.vector.scalar_tensor_tensor(
                            tp[:, :gw], psUb[:, :gw], 2.0,
                            tp[:, :gw],
                            mybir.AluOpType.mult,
                            mybir.AluOpType.add,
                        )
                    # Y' = W_s^T @ T'
                    psY = psyp.tile([128, GROUP], f32)
                    nc.tensor.matmul(
                        psY[:, :gw], wk_t[:, s * 128 : s * 128 + 128],
                        tp[:, :gw], start=True, stop=True,
                    )
                    ys = workp.tile([128, GROUP], f32, tag="ys")
                    nc.scalar.activation(
                        ys[:, :gw], psY[:, :gw],
                        mybir.ActivationFunctionType.Copy,
                    )
                    psT = pstp.tile([128, GROUP], f32)
                    for i in range((gw + 127) // 128):
                        wi = min(128, gw - 128 * i)
                        nc.tensor.transpose(
                            psT[:wi, i * 128 : i * 128 + 128],
                            ys[:, i * 128 : i * 128 + wi],
                            ident_t[:],
                        )
                    out_update(s, g, gw, psT)
                    # node-major T_s → shard dram for AllGather
                    if tks[s] is not None:
                        psN = pstp.tile([128, GROUP], f32, tag="psN")
                        for i in range((gw + 127) // 128):
                            wi = min(128, gw - 128 * i)
                            nc.tensor.transpose(
                                psN[:wi, i * 128 : i * 128 + 128],
                                tp[:, i * 128 : i * 128 + wi],
                                ident_t[:],
                            )
                        tn = workp.tile([128, GROUP], bf16, tag="tn")
                        if gw == GROUP:
                            nc.scalar.activation(
                                tn[:], psN[:],
                                mybir.ActivationFunctionType.Copy,
                            )
                        else:
                            for i in range((gw + 127) // 128):
                                wi = min(128, gw - 128 * i)
                                nc.scalar.activation(
                                    tn[:wi, i * 128 : i * 128 + 128],
                                    psN[:wi, i * 128 : i * 128 + 128],
                                    mybir.ActivationFunctionType.Copy,
                                )
                        for i in range((gw + 127) // 128):
                            wi = min(128, gw - 128 * i)
                            nc.scalar.dma_start(
                                tks[s][g * GROUP + i * 128
                                       : g * GROUP + i * 128 + wi, :],
                                tn[:wi, i * 128 : i * 128 + 128],
                            )
                if tks[s] is not None and not no_ag:
                    nc.gpsimd.collective_compute(
                        "AllGather",
                        mybir.AluOpType.bypass,
                        replica_groups=[list(range(p.cores))],
                        ins=[tks[s].ap().opt()],
                        outs=[tkf[s].ap().opt()],
                    )

            nc.sync.dma_start(out_d[:], out_acc[:])

    nc.compile()
    return nc


def _make_in_maps(p):
    iota_hi = np.broadcast_to(
        np.tile(np.arange(16, dtype=np.float32), UCH), (128, UCH * 16)
    ).astype(BF16)
    iota_lo = np.broadcast_to(
        np.tile(np.arange(8, dtype=np.float32), UCH), (128, UCH * 8)
    ).astype(BF16)
    ident = np.eye(128, dtype=np.float32)
    maps = []
    for c in range(p.cores):
        maps.append(
            {
                "x_full": p.x_full,
                "xt": p.xt[c],
                "weight": p.weight,
                "call": p.call[c],
                "idx": p.idx[c],
                "wcol": p.wcol[c],
                "dstl_hi": p.dstl_hi[c],
                "dstl_lo": p.dstl_lo[c],
                "iota_hi": iota_hi,
                "iota_lo": iota_lo,
                "ident": ident,
            }
        )
    return maps


_LAST_EXEC_NS = None


def run(x, filter_coeff, weight, bias, edge_w, src, dst, *, cores=8, nwin=4,
        trace=False, sim=False):
    global _LAST_EXEC_NS
    n, f = np.asarray(x).shape
    assert f == F
    k = np.asarray(weight).shape[0]
    p = _pack(x, filter_coeff, weight, edge_w, src, dst, n, cores, k, nwin)
    nc = _build(p)
    in_maps = _make_in_maps(p)

    if sim:
        from concourse.bass_interp import MultiCoreSim

        msim = MultiCoreSim(nc, cores)
        for c in range(cores):
            for name, arr in in_maps[c].items():
                msim.cores[c].tensor(name)[:] = arr
        msim.simulate()
        outs = [
            np.array(msim.cores[c].mem_tensor("out")) for c in range(cores)
        ]
    else:
        _install_ntff_hook()
        from concourse import bass_utils

        res = bass_utils.run_bass_kernel_spmd(
            nc, in_maps, core_ids=list(range(cores)), trace=trace
        )
        _LAST_EXEC_NS = res.exec_time_ns
        outs = [res.results[c]["out"] for c in range(cores)]

    nshard = n // cores
    shards = []
    for o in outs:
        # out is [128, ntiles*128] partition-major: o[p, t*128+f] = row t*128+p
        full_pad = (
            np.asarray(o)
            .reshape(128, p.ntiles, F)
            .transpose(1, 0, 2)
            .reshape(p.ntiles * 128, F)
        )
        shards.append(full_pad[:nshard])
    full = np.concatenate(shards, axis=0)
    return (full + np.asarray(bias, dtype=np.float32)[None, :]).astype(
        np.float32
    )


def kernel(x, filter_coeff, weight, bias, edge_w, src, dst):
    import os

    trace = bool(os.environ.get("KBENCH_TRACE"))
    return run(x, filter_coeff, weight, bias, edge_w, src, dst, trace=trace)


def last_exec_time_ns():
    return _LAST_EXEC_NS
